# revision 50
# baseline (speedup 1.0000x reference)
"""GAT (2-layer, DGL-style GATConv) on 8 Trainium2 NeuronCores.

Sharding: dst-node partition (graph parallel). The host relabels nodes
(degree-balanced snake deal over in-degree-sorted nodes, degree-sorted within
each core) so each core owns a contiguous block of N/8 dst nodes, and groups
each core's incoming edges into a node-major, per-tile-K-padded slot layout.
Per layer, on device:
  - per-core projection of its node block (PE matmuls against the
    host-concatenated [W | w_attn_l | w_attn_r] weight block),
  - AllGather of the per-shard gather table (rows = [el | feat]),
  - per-edge gather of src rows via indirect DMA (one 128-row descriptor
    batch per slot column), then node-major segment softmax + weighted
    aggregation as free-axis strided DVE/ACT ops (no max-subtraction needed:
    scores are O(1) so exp is safe, and softmax is shift-invariant).
Padded slots point at a per-shard dummy row with el = -1e9, feat = 0, so they
contribute exp(-inf)=0. Output rows are inverse-permuted on the host.

The program is split into four TileContexts (P0 / AG+E0 / P1 / AG+E1) so each
phase's DMA-completion semaphores reset before the 16-bit wait values overflow
(each edge phase alone runs ~1600 indirect DMAs x16 increments).
"""
import sys

sys.path.insert(0, "/opt/trn_rl_repo")

from contextlib import ExitStack

import os

import numpy as np

P = 128
WBUFS = int(os.environ.get("WBUFS", "3"))
SKIP_E0 = False
SKIP_E1 = False
SKIP_AG = False
SKIP_AG0 = False
SKIP_AG1 = False
GBUFS = int(os.environ.get("GBUFS", "3"))
NC = 8
SLOPE = 0.2


def _host_shard(src, dst, n_nodes):
    """Node permutation + per-core slot-packed edge index columns."""
    deg = np.bincount(dst, minlength=n_nodes)
    order = np.argsort(-deg, kind="stable")
    i = np.arange(n_nodes)
    r, j = i // NC, i % NC
    core_of_order = np.where(r % 2 == 0, j, NC - 1 - j)
    perm_c = [order[core_of_order == c] for c in range(NC)]
    nsh = n_nodes // NC
    vsh = nsh + 1  # +1 dummy row per shard
    newid = np.empty(n_nodes, np.int64)
    for c in range(NC):
        assert len(perm_c[c]) == nsh
        newid[perm_c[c]] = c * vsh + np.arange(nsh)

    ntt = (nsh + P - 1) // P
    degloc = np.zeros((NC, ntt * P), np.int64)
    for c in range(NC):
        degloc[c, :nsh] = deg[perm_c[c]]
    Kt = degloc.reshape(NC, ntt, P).max(axis=(0, 2))
    Kt = np.maximum(Kt, 1).astype(np.int64)

    tile_off = np.concatenate([[0], np.cumsum(Kt * P)])
    slots = int(tile_off[-1])
    src_n = newid[src]
    dst_n = newid[dst]
    ecore = dst_n // vsh
    dloc = (dst_n % vsh).astype(np.int64)

    idx2 = np.empty((NC, P, int(Kt.sum())), np.int32)
    cumK = np.concatenate([[0], np.cumsum(Kt)]).astype(int)
    for c in range(NC):
        m = ecore == c
        es, dl = src_n[m], dloc[m]
        o = np.argsort(dl, kind="stable")
        es, dl = es[o], dl[o]
        first = np.searchsorted(dl, dl, side="left")
        k = np.arange(len(dl)) - first
        t, p = dl // P, dl % P
        flat = tile_off[t] + p * Kt[t] + k
        eidx = np.full(slots, c * vsh + nsh, np.int64)
        eidx[flat] = es
        for t2 in range(ntt):
            blk = eidx[tile_off[t2]:tile_off[t2 + 1]].reshape(P, int(Kt[t2]))
            idx2[c, :, cumK[t2]:cumK[t2 + 1]] = blk

    # ---- layer-1 edge stream for dma_gather/scatter: sorted by (src-window,
    # round) where round = per-(window, dst) occurrence index. Each (window,
    # round) block has all-distinct dst, so its scatter-add is collision-free.
    WIN = 32768
    V = NC * vsh
    nwin = (V + WIN - 1) // WIN
    dummies = []
    for w in range(nwin):
        cands = [c * vsh + nsh for c in range(NC)
                 if w * WIN <= c * vsh + nsh < min(V, (w + 1) * WIN)]
        dummies.append(cands[0])
    per_core = []
    for c in range(NC):
        m = ecore == c
        es, dl = src_n[m], dloc[m]
        w = es // WIN
        key = w * 20000 + dl
        o = np.argsort(key, kind="stable")
        es, dl, w, key = es[o], dl[o], w[o], key[o]
        first = np.searchsorted(key, key, side="left")
        r = np.arange(len(key)) - first
        per_core.append((es, dl, w, r))
    blocks = []  # (window, round, padded block size)
    for w in range(nwin):
        maxr = max(int(r[wv == w].max()) + 1 if (wv == w).any() else 0
                   for _, _, wv, r in per_core)
        for ri in range(maxr):
            mx = max(int(((wv == w) & (rv == ri)).sum())
                     for _, _, wv, rv in per_core)
            blocks.append((w, ri, ((mx + P - 1) // P) * P))
    Lc = int(sum(b[2] for b in blocks))
    gsrc = np.empty((NC, Lc), np.int64)
    gdst = np.empty((NC, Lc), np.int64)
    off = 0
    for w, ri, sz in blocks:
        for c in range(NC):
            es, dl, wv, rv = per_core[c]
            mm = (wv == w) & (rv == ri)
            cnt = int(mm.sum())
            gsrc[c, off:off + cnt] = es[mm]
            gdst[c, off:off + cnt] = dl[mm]
            gsrc[c, off + cnt:off + sz] = dummies[w]
            gdst[c, off + cnt:off + sz] = nsh
        off += sz

    def wrap16(a):  # [NC, L] -> [NC, 128, L//16] int16, wrapped + replicated
        L = a.shape[1]
        out = np.empty((NC, P, L // 16), np.int16)
        for c in range(NC):
            blk = a[c].reshape(L // 16, 16).T.astype(np.int16)
            out[c] = np.tile(blk, (8, 1))
        return out

    grel = gsrc.copy()
    off = 0
    for w, ri, sz in blocks:
        grel[:, off:off + sz] -= w * WIN
        off += sz
    gidx16 = wrap16(grel)
    dloc16 = wrap16(gdst)
    return (perm_c, Kt, idx2, nsh, vsh, ntt, gidx16, dloc16,
            tuple((w, ri, sz) for w, ri, sz in blocks))


def _build_program(n_in, h0, d0, h1, c1, Kt, nsh, vsh, ntt, blocks):
    import concourse.bass as bass
    import concourse.mybir as mybir
    from concourse import tile, bacc
    from concourse.masks import make_identity

    f0 = h0 * d0          # 128
    f1 = h1 * c1          # 40
    # bf16 table rows: [el_hi(h) | el_lo(h) | feat(f) | pad]. el is carried as
    # a bf16 hi/lo split so the attention scores keep ~f32 precision while the
    # features ride in bf16 (halves gather + AllGather bytes). Rows padded to
    # 16-element (32B) multiples: unaligned rows crash large AllGathers.
    row0 = ((2 * h0 + f0 + 15) // 16) * 16   # 144
    # layer-1 rows are fetched with dma_gather: stride must be a 256B
    # multiple -> 128 bf16 columns.
    row1 = 128
    V = NC * vsh
    ckt = int(Kt.sum())
    WIN = 32768
    Lc = int(sum(b[2] for b in blocks))
    # HW-probed per-call limits: gathers at 1024 idx, scatters at 512.
    CHG = 1024
    CHS = 512
    ACC = 64  # f32 accumulator row: [den | 40 feat | 23 junk]
    AF = mybir.ActivationFunctionType
    OP = mybir.AluOpType
    dt = mybir.dt

    nc = bacc.Bacc()
    xT = nc.declare_dram_parameter("xT", [n_in, nsh], dt.bfloat16, isOutput=False)
    eidx = nc.declare_dram_parameter("eidx", [P, ckt], dt.int32, isOutput=False)
    gidx16 = nc.declare_dram_parameter("gidx16", [P, Lc // 16], dt.int16, isOutput=False)
    dloc16 = nc.declare_dram_parameter("dloc16", [P, Lc // 16], dt.int16, isOutput=False)
    w0cat = nc.declare_dram_parameter("w0cat", [n_in, f0 + 2 * h0], dt.bfloat16, isOutput=False)
    w1cat = nc.declare_dram_parameter("w1cat", [f0, f1 + 2 * h1], dt.float32, isOutput=False)
    out_d = nc.declare_dram_parameter("out", [nsh, c1], dt.float32, isOutput=True)

    tab0_sh = nc.dram_tensor("tab0_sh", [vsh, row0], dt.bfloat16)
    tab0 = nc.dram_tensor("tab0", [V, row0], dt.bfloat16, addr_space="Shared")
    tab1_sh = nc.dram_tensor("tab1_sh", [vsh, row1], dt.bfloat16)
    tab1 = nc.dram_tensor("tab1", [V, row1], dt.bfloat16, addr_space="Shared")
    er1_tab = nc.dram_tensor("er1_tab", [vsh, row1], dt.bfloat16)
    acc_ab = [
        nc.dram_tensor("acc_a", [vsh, ACC], dt.float32),
        nc.dram_tensor("acc_b", [vsh, ACC], dt.float32),
    ]

    cumK = np.concatenate([[0], np.cumsum(Kt)]).astype(int)
    KCH = n_in // P

    with ExitStack() as ctx:
        idx_sb = ctx.enter_context(nc.sbuf_tensor([P, ckt], dt.int32))
        gidx_sb = ctx.enter_context(nc.sbuf_tensor([P, Lc // 16], dt.int16))
        dloc_sb = ctx.enter_context(nc.sbuf_tensor([P, Lc // 16], dt.int16))
        er0_sb = ctx.enter_context(nc.sbuf_tensor([P, ntt, h0], dt.float32))
        w1_sb = ctx.enter_context(nc.sbuf_tensor([P, f1 + 2 * h1], dt.float32))
        ident = ctx.enter_context(nc.sbuf_tensor([P, P], dt.float32))

        # ---------- context 1: preamble + P0 projection ----------
        with tile.TileContext(nc) as tc:
            with (
                tc.tile_pool(name="work", bufs=WBUFS) as wp,
                tc.tile_pool(name="psum", bufs=2, space="PSUM") as psp,
                tc.tile_pool(name="wconst", bufs=1) as wc,
            ):
                w0_sb = wc.tile([P, KCH, f0 + 2 * h0], dt.bfloat16)
                nc.sync.dma_start(out=idx_sb[:], in_=eidx[:])
                nc.sync.dma_start(out=gidx_sb[:], in_=gidx16[:])
                nc.sync.dma_start(out=dloc_sb[:], in_=dloc16[:])
                nc.sync.dma_start(
                    out=w0_sb[:], in_=w0cat[:].rearrange("(c p) w -> p c w", p=P)
                )
                nc.sync.dma_start(out=w1_sb[:], in_=w1cat[:])
                nc.gpsimd.memset(er0_sb[:], 0.0)
                make_identity(nc, ident[:])

                zacc = wc.tile([P, ACC], dt.float32)
                nc.gpsimd.memset(zacc[:], 0.0)
                for t in range(ntt + 1):
                    nn = min(P, vsh - t * P)
                    if nn <= 0:
                        break
                    for acc in acc_ab:
                        nc.sync.dma_start(
                            out=acc[t * P:t * P + nn, :], in_=zacc[:nn, :]
                        )

                drow = wp.tile([1, row0], dt.bfloat16, tag="drow")
                nc.gpsimd.memset(drow[:], 0.0)
                nc.gpsimd.memset(drow[:, :2 * h0], -1e9)
                nc.sync.dma_start(out=tab0_sh[nsh:nsh + 1, :], in_=drow[:])
                drow1 = wp.tile([1, row1], dt.bfloat16, tag="drow1")
                nc.gpsimd.memset(drow1[:], 0.0)
                nc.gpsimd.memset(drow1[:, :2 * h1], -1e9)
                nc.sync.dma_start(out=tab1_sh[nsh:nsh + 1, :], in_=drow1[:])
                drow2 = wp.tile([1, row1], dt.bfloat16, tag="drow2")
                nc.gpsimd.memset(drow2[:], 0.0)
                nc.sync.dma_start(out=er1_tab[nsh:nsh + 1, :], in_=drow2[:])

                for t in range(ntt):
                    nn = min(P, nsh - t * P)
                    ps = psp.tile([P, f0 + 2 * h0], dt.float32, tag="proj")
                    xk = wp.tile([P, KCH, P], dt.bfloat16, tag="xk")
                    nc.sync.dma_start(
                        out=xk[:, :, :nn],
                        in_=xT[:, t * P:t * P + nn].rearrange("(c p) n -> p c n", p=P),
                    )
                    for kc in range(KCH):
                        nc.tensor.matmul(
                            ps[:nn, :], lhsT=xk[:, kc, :nn], rhs=w0_sb[:, kc, :],
                            start=(kc == 0), stop=(kc == KCH - 1),
                        )
                    row = wp.tile([P, row0], dt.bfloat16, tag="row")
                    nc.vector.tensor_copy(row[:nn, :h0], ps[:nn, f0:f0 + h0])
                    hi32 = wp.tile([P, h0], dt.float32, tag="hi32")
                    nc.vector.tensor_copy(hi32[:nn, :], row[:nn, :h0])
                    lo32 = wp.tile([P, h0], dt.float32, tag="lo32")
                    nc.vector.tensor_tensor(
                        out=lo32[:nn, :], in0=ps[:nn, f0:f0 + h0], in1=hi32[:nn, :],
                        op=OP.subtract,
                    )
                    nc.vector.tensor_copy(row[:nn, h0:2 * h0], lo32[:nn, :])
                    nc.scalar.activation(row[:nn, 2 * h0:2 * h0 + f0], ps[:nn, :f0], AF.Copy)
                    nc.vector.tensor_copy(er0_sb[:nn, t, :], ps[:nn, f0 + h0:])
                    nc.sync.dma_start(out=tab0_sh[t * P:t * P + nn, :], in_=row[:nn, :])

        def edge_phase(tc, wp, gp, tab, hh, dd, row_w, er_sb, sink, tagp):
            ff = hh * dd
            for t in range(ntt):
                K = int(Kt[t])
                nn = min(P, nsh - t * P)
                g = gp.tile([P, K, row_w], dt.bfloat16, tag="G" + tagp)
                skip = SKIP_E0 if tagp == "0" else SKIP_E1
                if skip:
                    nc.gpsimd.memset(g[:], 0.001)
                else:
                    for k in range(K):
                        nc.gpsimd.indirect_dma_start(
                            out=g[:, k, :], out_offset=None, in_=tab[:],
                            in_offset=bass.IndirectOffsetOnAxis(
                                ap=idx_sb[:, cumK[t] + k:cumK[t] + k + 1], axis=0
                            ),
                        )
                # e = (el_hi + el_lo) + er  (hi/lo bf16 split reconstructs el)
                e_sb = wp.tile([P, hh, K], dt.float32, tag="e" + tagp)
                nc.vector.tensor_tensor(
                    out=e_sb[:],
                    in0=g[:, :, 0:hh].rearrange("p k h -> p h k"),
                    in1=g[:, :, hh:2 * hh].rearrange("p k h -> p h k"),
                    op=OP.add,
                )
                nc.vector.tensor_tensor(
                    out=e_sb[:], in0=e_sb[:],
                    in1=er_sb[:, t, :].to_broadcast([P, hh, K]), op=OP.add,
                )
                lk = wp.tile([P, hh, K], dt.float32, tag="lk" + tagp)
                nc.vector.tensor_scalar_mul(lk[:], e_sb[:], SLOPE)
                nc.vector.tensor_tensor(out=e_sb[:], in0=e_sb[:], in1=lk[:], op=OP.max)
                nc.scalar.activation(e_sb[:], e_sb[:], AF.Exp)
                den = wp.tile([P, hh], dt.float32, tag="den" + tagp)
                nc.vector.tensor_reduce(den[:], e_sb[:], axis=mybir.AxisListType.X, op=OP.add)
                nc.vector.tensor_scalar_max(den[:], den[:], 1e-9)
                rec = wp.tile([P, hh], dt.float32, tag="rec" + tagp)
                nc.vector.reciprocal(rec[:], den[:])
                # alpha-weight the gathered features in place (bf16)
                fslice = g[:, :, 2 * hh:2 * hh + ff]
                nc.vector.tensor_tensor(
                    out=fslice.rearrange("p k (h d) -> p k h d", h=hh),
                    in0=fslice.rearrange("p k (h d) -> p k h d", h=hh),
                    in1=e_sb[:].rearrange("p h k -> p k h").to_broadcast([P, K, hh, dd]),
                    op=OP.mult,
                )
                orw = wp.tile([P, ff], dt.float32, tag="oraw" + tagp)
                nc.vector.tensor_reduce(
                    orw[:], fslice.rearrange("p k f -> p f k"),
                    axis=mybir.AxisListType.X, op=OP.add,
                )
                sink(t, nn, orw, rec, hh, dd, wp)

        # ---------- context 2: AllGather0 + E0 + fused P1 ----------
        with tile.TileContext(nc) as tc:
            with (
                tc.tile_pool(name="work", bufs=WBUFS) as wp,
                tc.tile_pool(name="gbuf", bufs=GBUFS) as gp,
                tc.tile_pool(name="psum1", bufs=2, space="PSUM") as psp1,
            ):
                if not SKIP_AG0:
                    nc.gpsimd.collective_compute(
                        "AllGather", OP.bypass, ins=[tab0_sh[:]], outs=[tab0[:]],
                        replica_groups=[list(range(NC))],
                    )
                else:
                    st = wp.tile([P, row0], dt.bfloat16, tag="stg")
                    for tt2 in range(ntt):
                        nnn = min(P, nsh - tt2 * P)
                        nc.sync.dma_start(out=st[:nnn, :], in_=tab0_sh[tt2*P:tt2*P+nnn, :])
                        nc.sync.dma_start(out=tab0[tt2*P:tt2*P+nnn, :], in_=st[:nnn, :])

                def sink0(t, nn, orw, rec, hh, dd, wp):
                    # h = elu(alpha-normalized aggregate), then project to the
                    # layer-1 table row immediately (P1 fused into E0).
                    x0 = wp.tile([P, f0], dt.float32, tag="x0")
                    nc.vector.tensor_tensor(
                        out=x0[:].rearrange("p (h d) -> p h d", h=hh),
                        in0=orw[:].rearrange("p (h d) -> p h d", h=hh),
                        in1=rec[:].to_broadcast([P, hh, dd]),
                        op=OP.mult,
                    )
                    relu = wp.tile([P, f0], dt.float32, tag="relu")
                    nc.vector.tensor_scalar_max(relu[:], x0[:], 0.0)
                    mneg = wp.tile([P, f0], dt.float32, tag="mneg")
                    nc.vector.tensor_scalar_min(mneg[:], x0[:], 0.0)
                    nc.scalar.activation(mneg[:], mneg[:], AF.Exp)
                    nc.vector.tensor_scalar(
                        out=mneg[:], in0=mneg[:], scalar1=-1.0, scalar2=0.0,
                        op0=OP.add, op1=OP.min,
                    )
                    hsb = wp.tile([P, f0], dt.float32, tag="hsb")
                    nc.vector.tensor_tensor(out=hsb[:], in0=relu[:], in1=mneg[:], op=OP.add)
                    hT_ps = psp1.tile([P, P], dt.float32, tag="hT")
                    nc.tensor.transpose(out=hT_ps[:], in_=hsb[:], identity=ident[:])
                    hT = wp.tile([P, P], dt.float32, tag="hTsb")
                    nc.vector.tensor_copy(hT[:], hT_ps[:])
                    ps1 = psp1.tile([P, f1 + 2 * h1], dt.float32, tag="proj1")
                    nc.tensor.matmul(
                        ps1[:nn, :], lhsT=hT[:, :nn], rhs=w1_sb[:], start=True, stop=True
                    )
                    row = wp.tile([P, row1], dt.bfloat16, tag="row1")
                    nc.vector.tensor_copy(row[:nn, :h1], ps1[:nn, f1:f1 + h1])
                    hi1 = wp.tile([P, h1], dt.float32, tag="hi1")
                    nc.vector.tensor_copy(hi1[:nn, :], row[:nn, :h1])
                    lo1 = wp.tile([P, h1], dt.float32, tag="lo1")
                    nc.vector.tensor_tensor(
                        out=lo1[:nn, :], in0=ps1[:nn, f1:f1 + h1], in1=hi1[:nn, :],
                        op=OP.subtract,
                    )
                    nc.vector.tensor_copy(row[:nn, h1:2 * h1], lo1[:nn, :])
                    nc.scalar.activation(row[:nn, 2 * h1:2 * h1 + f1], ps1[:nn, :f1], AF.Copy)
                    nc.sync.dma_start(out=tab1_sh[t * P:t * P + nn, :], in_=row[:nn, :])
                    errow = wp.tile([P, row1], dt.bfloat16, tag="er1row")
                    nc.vector.tensor_copy(errow[:nn, :h1], ps1[:nn, f1 + h1:f1 + 2 * h1])
                    ehi = wp.tile([P, h1], dt.float32, tag="ehi1")
                    nc.vector.tensor_copy(ehi[:nn, :], errow[:nn, :h1])
                    elo = wp.tile([P, h1], dt.float32, tag="elo1")
                    nc.vector.tensor_tensor(
                        out=elo[:nn, :], in0=ps1[:nn, f1 + h1:f1 + 2 * h1],
                        in1=ehi[:nn, :], op=OP.subtract,
                    )
                    nc.vector.tensor_copy(errow[:nn, h1:2 * h1], elo[:nn, :])
                    nc.sync.dma_start(out=er1_tab[t * P:t * P + nn, :], in_=errow[:nn, :])

                edge_phase(tc, wp, gp, tab0, h0, d0, row0, er0_sb, sink0, "0")

        # ---------- context 4: AllGather1 + E1 (windowed dma_gather + round-
        # structured collision-free dma_scatter_add, parity-split accumulators)
        with tile.TileContext(nc) as tc:
            with (
                tc.tile_pool(name="work", bufs=2) as wp,
                tc.tile_pool(name="gbuf", bufs=2) as gp,
            ):
                nc.gpsimd.collective_compute(
                    "AllGather", OP.bypass, ins=[tab1_sh[:]], outs=[tab1[:]],
                    replica_groups=[list(range(NC))],
                )

                estart = 0
                for w, ri, sz in blocks:
                    wbase = w * WIN
                    wlen = min(WIN, V - wbase)
                    acc = acc_ab[ri % 2]
                    done = 0
                    while done < sz:
                        cs = min(CHG, sz - done)
                        Gc = cs // P
                        c16 = estart // 16
                        g = gp.tile([P, CHG // P, row1], dt.bfloat16, tag="g1")
                        nc.gpsimd.dma_gather(
                            g[:, :Gc, :], tab1[wbase:wbase + wlen, :],
                            gidx_sb[:, c16:c16 + cs // 16], cs, cs, row1,
                        )
                        ger = gp.tile([P, CHG // P, row1], dt.bfloat16, tag="ger1")
                        nc.gpsimd.dma_gather(
                            ger[:, :Gc, :], er1_tab[:, :],
                            dloc_sb[:, c16:c16 + cs // 16], cs, cs, row1,
                        )
                        e1 = wp.tile([P, CHG // P, h1], dt.float32, tag="e1")
                        nc.vector.tensor_tensor(
                            out=e1[:, :Gc, :], in0=g[:, :Gc, 0:h1],
                            in1=g[:, :Gc, h1:2 * h1], op=OP.add,
                        )
                        nc.vector.tensor_tensor(
                            out=e1[:, :Gc, :], in0=e1[:, :Gc, :],
                            in1=ger[:, :Gc, 0:h1], op=OP.add,
                        )
                        nc.vector.tensor_tensor(
                            out=e1[:, :Gc, :], in0=e1[:, :Gc, :],
                            in1=ger[:, :Gc, h1:2 * h1], op=OP.add,
                        )
                        lk1 = wp.tile([P, CHG // P, h1], dt.float32, tag="lk1")
                        nc.vector.tensor_scalar_mul(lk1[:, :Gc, :], e1[:, :Gc, :], SLOPE)
                        nc.vector.tensor_tensor(
                            out=e1[:, :Gc, :], in0=e1[:, :Gc, :], in1=lk1[:, :Gc, :],
                            op=OP.max,
                        )
                        nc.scalar.activation(e1[:, :Gc, :], e1[:, :Gc, :], AF.Exp)
                        msg = wp.tile([P, CHG // P, ACC], dt.float32, tag="msg1")
                        nc.vector.tensor_copy(msg[:, :Gc, 0:1], e1[:, :Gc, :])
                        nc.vector.tensor_tensor(
                            out=msg[:, :Gc, 1:1 + f1],
                            in0=g[:, :Gc, 2 * h1:2 * h1 + f1],
                            in1=e1[:, :Gc, :].to_broadcast([P, Gc, f1]),
                            op=OP.mult,
                        )
                        nc.vector.tensor_copy(
                            msg[:, :Gc, 1 + f1:],
                            e1[:, :Gc, :].to_broadcast([P, Gc, ACC - 1 - f1]),
                        )
                        sdone = 0
                        while sdone < cs:
                            ss = min(CHS, cs - sdone)
                            nc.gpsimd.dma_scatter_add(
                                acc[:, :],
                                msg[:, sdone // P:(sdone + ss) // P, :],
                                dloc_sb[:, (estart + sdone) // 16:
                                        (estart + sdone + ss) // 16],
                                ss, ss, ACC,
                            )
                            sdone += ss
                        done += cs
                        estart += cs

                # out = (acc_a + acc_b feat) / (acc_a + acc_b den)
                for t in range(ntt):
                    nn = min(P, nsh - t * P)
                    accl = wp.tile([P, ACC], dt.float32, tag="accl")
                    nc.sync.dma_start(out=accl[:nn, :], in_=acc_ab[0][t * P:t * P + nn, :])
                    accr = wp.tile([P, ACC], dt.float32, tag="accr")
                    nc.sync.dma_start(out=accr[:nn, :], in_=acc_ab[1][t * P:t * P + nn, :])
                    nc.vector.tensor_tensor(
                        out=accl[:nn, :], in0=accl[:nn, :], in1=accr[:nn, :], op=OP.add
                    )
                    den = wp.tile([P, 1], dt.float32, tag="den1")
                    nc.vector.tensor_scalar_max(den[:nn, :], accl[:nn, 0:1], 1e-9)
                    rec = wp.tile([P, 1], dt.float32, tag="rec1")
                    nc.vector.reciprocal(rec[:nn, :], den[:nn, :])
                    ov = wp.tile([P, f1], dt.float32, tag="ov")
                    nc.vector.tensor_tensor(
                        out=ov[:nn, :], in0=accl[:nn, 1:1 + f1],
                        in1=rec[:nn, :].to_broadcast([nn, f1]), op=OP.mult,
                    )
                    nc.sync.dma_start(out=out_d[t * P:t * P + nn, :], in_=ov[:nn, :])

    nc.compile()
    return nc


_CACHE = {}


def build_cached(n_in, h0, d0, h1, c1, Kt, nsh, vsh, ntt, blocks):
    key = (n_in, h0, d0, h1, c1, nsh, vsh, ntt, tuple(Kt.tolist()), blocks)
    if key not in _CACHE:
        _CACHE[key] = _build_program(n_in, h0, d0, h1, c1, Kt, nsh, vsh, ntt, blocks)
    return _CACHE[key]


def make_in_maps(x, W0, al0, ar0, W1, al1, ar1, perm_c, idx2, gidx16, dloc16):
    n_in = x.shape[1]
    h0, d0 = al0.shape
    h1, c1 = al1.shape
    wl0 = np.einsum("ihd,hd->ih", W0.reshape(n_in, h0, d0), al0).astype(np.float32)
    wr0 = np.einsum("ihd,hd->ih", W0.reshape(n_in, h0, d0), ar0).astype(np.float32)
    wl1 = np.einsum("ihd,hd->ih", W1.reshape(h0 * d0, h1, c1), al1).astype(np.float32)
    wr1 = np.einsum("ihd,hd->ih", W1.reshape(h0 * d0, h1, c1), ar1).astype(np.float32)
    import ml_dtypes

    bf16 = ml_dtypes.bfloat16
    w0cat = np.ascontiguousarray(
        np.concatenate([W0, wl0, wr0], axis=1)
    ).astype(bf16)
    w1cat = np.ascontiguousarray(np.concatenate([W1, wl1, wr1], axis=1))
    return [
        {
            "xT": np.ascontiguousarray(x[perm_c[c]].T).astype(bf16),
            "eidx": np.ascontiguousarray(idx2[c]),
            "gidx16": np.ascontiguousarray(gidx16[c]),
            "dloc16": np.ascontiguousarray(dloc16[c]),
            "w0cat": w0cat,
            "w1cat": w1cat,
        }
        for c in range(NC)
    ]


LAST_EXEC_NS = None
LAST_MEAN_EXEC_NS = None


def kernel(x, src, dst, W0, al0, ar0, W1, al1, ar1):
    x = np.asarray(x, np.float32)
    src = np.asarray(src, np.int32)
    dst = np.asarray(dst, np.int32)
    W0 = np.asarray(W0, np.float32)
    al0 = np.asarray(al0, np.float32)
    ar0 = np.asarray(ar0, np.float32)
    W1 = np.asarray(W1, np.float32)
    al1 = np.asarray(al1, np.float32)
    ar1 = np.asarray(ar1, np.float32)

    n_nodes, n_in = x.shape
    h0, d0 = al0.shape
    h1, c1 = al1.shape

    (perm_c, Kt, idx2, nsh, vsh, ntt, gidx16, dloc16, blocks) = _host_shard(
        src, dst, n_nodes
    )
    nc = build_cached(n_in, h0, d0, h1, c1, Kt, nsh, vsh, ntt, blocks)
    in_maps = make_in_maps(
        x, W0, al0, ar0, W1, al1, ar1, perm_c, idx2, gidx16, dloc16
    )

    from concourse.bass_utils import run_bass_kernel_spmd

    trace = bool(int(os.environ.get("KERNEL_TRACE", "0")))
    res = run_bass_kernel_spmd(nc, in_maps, list(range(NC)), trace=trace)
    global LAST_EXEC_NS, LAST_MEAN_EXEC_NS
    LAST_EXEC_NS = res.exec_time_ns
    LAST_MEAN_EXEC_NS = res.mean_exec_time_ns
    out = np.empty((n_nodes, c1), np.float32)
    for c in range(NC):
        out[perm_c[c]] = res.results[c]["out"]
    return out



# revision 53
# speedup vs baseline: 1.6788x; 1.6788x over previous
"""GAT (2-layer, DGL-style GATConv) on 8 Trainium2 NeuronCores.

Sharding: dst-node partition (graph parallel). The host relabels nodes
(degree-balanced snake deal over in-degree-sorted nodes, degree-sorted within
each core) so each core owns a contiguous block of N/8 dst nodes, and groups
each core's incoming edges into a node-major, per-tile-K-padded slot layout.
Per layer, on device:
  - per-core projection of its node block (PE matmuls against the
    host-concatenated [W | w_attn_l | w_attn_r] weight block),
  - AllGather of the per-shard gather table (rows = [el | feat]),
  - per-edge gather of src rows via indirect DMA (one 128-row descriptor
    batch per slot column), then node-major segment softmax + weighted
    aggregation as free-axis strided DVE/ACT ops (no max-subtraction needed:
    scores are O(1) so exp is safe, and softmax is shift-invariant).
Padded slots point at a per-shard dummy row with el = -1e9, feat = 0, so they
contribute exp(-inf)=0. Output rows are inverse-permuted on the host.

The program is split into four TileContexts (P0 / AG+E0 / P1 / AG+E1) so each
phase's DMA-completion semaphores reset before the 16-bit wait values overflow
(each edge phase alone runs ~1600 indirect DMAs x16 increments).
"""
import sys

sys.path.insert(0, "/opt/trn_rl_repo")

from contextlib import ExitStack

import os

import numpy as np

P = 128
WBUFS = int(os.environ.get("WBUFS", "3"))
SKIP_E0 = False
SKIP_E1 = False
SKIP_AG = False
SKIP_AG0 = False
SKIP_AG1 = False
GBUFS = int(os.environ.get("GBUFS", "3"))
NC = 8
SLOPE = 0.2


def _host_shard(src, dst, n_nodes):
    """Node permutation + per-core slot-packed edge index columns."""
    deg = np.bincount(dst, minlength=n_nodes)
    order = np.argsort(-deg, kind="stable")
    i = np.arange(n_nodes)
    r, j = i // NC, i % NC
    core_of_order = np.where(r % 2 == 0, j, NC - 1 - j)
    perm_c = [order[core_of_order == c] for c in range(NC)]
    nsh = n_nodes // NC
    vsh = nsh + 1  # +1 dummy row per shard
    newid = np.empty(n_nodes, np.int64)
    for c in range(NC):
        assert len(perm_c[c]) == nsh
        newid[perm_c[c]] = c * vsh + np.arange(nsh)

    ntt = (nsh + P - 1) // P
    degloc = np.zeros((NC, ntt * P), np.int64)
    for c in range(NC):
        degloc[c, :nsh] = deg[perm_c[c]]
    Kt = degloc.reshape(NC, ntt, P).max(axis=(0, 2))
    Kt = np.maximum(Kt, 1).astype(np.int64)

    tile_off = np.concatenate([[0], np.cumsum(Kt * P)])
    slots = int(tile_off[-1])
    src_n = newid[src]
    dst_n = newid[dst]
    ecore = dst_n // vsh
    dloc = (dst_n % vsh).astype(np.int64)

    idx2 = np.empty((NC, P, int(Kt.sum())), np.int32)
    cumK = np.concatenate([[0], np.cumsum(Kt)]).astype(int)
    for c in range(NC):
        m = ecore == c
        es, dl = src_n[m], dloc[m]
        o = np.argsort(dl, kind="stable")
        es, dl = es[o], dl[o]
        first = np.searchsorted(dl, dl, side="left")
        k = np.arange(len(dl)) - first
        t, p = dl // P, dl % P
        flat = tile_off[t] + p * Kt[t] + k
        eidx = np.full(slots, c * vsh + nsh, np.int64)
        eidx[flat] = es
        for t2 in range(ntt):
            blk = eidx[tile_off[t2]:tile_off[t2 + 1]].reshape(P, int(Kt[t2]))
            idx2[c, :, cumK[t2]:cumK[t2 + 1]] = blk
    return perm_c, Kt, idx2, nsh, vsh, ntt


def _build_program(n_in, h0, d0, h1, c1, Kt, nsh, vsh, ntt):
    import concourse.bass as bass
    import concourse.mybir as mybir
    from concourse import tile, bacc
    from concourse.masks import make_identity

    f0 = h0 * d0          # 128
    f1 = h1 * c1          # 40
    # bf16 table rows: [el_hi(h) | el_lo(h) | feat(f) | pad]. el is carried as
    # a bf16 hi/lo split so the attention scores keep ~f32 precision while the
    # features ride in bf16 (halves gather + AllGather bytes). Rows padded to
    # 16-element (32B) multiples: unaligned rows crash large AllGathers.
    row0 = ((2 * h0 + f0 + 15) // 16) * 16   # 144
    row1 = ((2 * h1 + f1 + 15) // 16) * 16   # 48
    V = NC * vsh
    ckt = int(Kt.sum())
    AF = mybir.ActivationFunctionType
    OP = mybir.AluOpType
    dt = mybir.dt

    nc = bacc.Bacc()
    xT = nc.declare_dram_parameter("xT", [n_in, nsh], dt.bfloat16, isOutput=False)
    eidx = nc.declare_dram_parameter("eidx", [P, ckt], dt.int32, isOutput=False)
    w0cat = nc.declare_dram_parameter("w0cat", [n_in, f0 + 2 * h0], dt.bfloat16, isOutput=False)
    w1cat = nc.declare_dram_parameter("w1cat", [f0, f1 + 2 * h1], dt.float32, isOutput=False)
    out_d = nc.declare_dram_parameter("out", [nsh, c1], dt.float32, isOutput=True)

    tab0_sh = nc.dram_tensor("tab0_sh", [vsh, row0], dt.bfloat16)
    tab0 = nc.dram_tensor("tab0", [V, row0], dt.bfloat16, addr_space="Shared")
    tab1_sh = nc.dram_tensor("tab1_sh", [vsh, row1], dt.bfloat16)
    tab1 = nc.dram_tensor("tab1", [V, row1], dt.bfloat16, addr_space="Shared")

    cumK = np.concatenate([[0], np.cumsum(Kt)]).astype(int)
    KCH = n_in // P

    with ExitStack() as ctx:
        idx_sb = ctx.enter_context(nc.sbuf_tensor([P, ckt], dt.int32))
        er0_sb = ctx.enter_context(nc.sbuf_tensor([P, ntt, h0], dt.float32))
        er1_sb = ctx.enter_context(nc.sbuf_tensor([P, ntt, h1], dt.float32))
        w1_sb = ctx.enter_context(nc.sbuf_tensor([P, f1 + 2 * h1], dt.float32))
        ident = ctx.enter_context(nc.sbuf_tensor([P, P], dt.float32))

        def edge_phase(tc, wp, gp, tab, hh, dd, row_w, er_sb, sink, tagp):
            ff = hh * dd
            for t in range(ntt):
                K = int(Kt[t])
                nn = min(P, nsh - t * P)
                g = gp.tile([P, K, row_w], dt.bfloat16, tag="G" + tagp)
                skip = SKIP_E0 if tagp == "0" else SKIP_E1
                if skip:
                    nc.gpsimd.memset(g[:], 0.001)
                else:
                    for k in range(K):
                        nc.gpsimd.indirect_dma_start(
                            out=g[:, k, :], out_offset=None, in_=tab[:],
                            in_offset=bass.IndirectOffsetOnAxis(
                                ap=idx_sb[:, cumK[t] + k:cumK[t] + k + 1], axis=0
                            ),
                        )
                # e = (el_hi + el_lo) + er  (hi/lo bf16 split reconstructs el)
                e_sb = wp.tile([P, hh, K], dt.float32, tag="e" + tagp)
                nc.vector.tensor_tensor(
                    out=e_sb[:],
                    in0=g[:, :, 0:hh].rearrange("p k h -> p h k"),
                    in1=g[:, :, hh:2 * hh].rearrange("p k h -> p h k"),
                    op=OP.add,
                )
                nc.vector.tensor_tensor(
                    out=e_sb[:], in0=e_sb[:],
                    in1=er_sb[:, t, :].to_broadcast([P, hh, K]), op=OP.add,
                )
                lk = wp.tile([P, hh, K], dt.float32, tag="lk" + tagp)
                nc.vector.tensor_scalar_mul(lk[:], e_sb[:], SLOPE)
                nc.vector.tensor_tensor(out=e_sb[:], in0=e_sb[:], in1=lk[:], op=OP.max)
                nc.scalar.activation(e_sb[:], e_sb[:], AF.Exp)
                den = wp.tile([P, hh], dt.float32, tag="den" + tagp)
                nc.vector.tensor_reduce(den[:], e_sb[:], axis=mybir.AxisListType.X, op=OP.add)
                nc.vector.tensor_scalar_max(den[:], den[:], 1e-9)
                rec = wp.tile([P, hh], dt.float32, tag="rec" + tagp)
                nc.vector.reciprocal(rec[:], den[:])
                # alpha-weight the gathered features in place (bf16)
                fslice = g[:, :, 2 * hh:2 * hh + ff]
                nc.vector.tensor_tensor(
                    out=fslice.rearrange("p k (h d) -> p k h d", h=hh),
                    in0=fslice.rearrange("p k (h d) -> p k h d", h=hh),
                    in1=e_sb[:].rearrange("p h k -> p k h").to_broadcast([P, K, hh, dd]),
                    op=OP.mult,
                )
                orw = wp.tile([P, ff], dt.float32, tag="oraw" + tagp)
                nc.vector.tensor_reduce(
                    orw[:], fslice.rearrange("p k f -> p f k"),
                    axis=mybir.AxisListType.X, op=OP.add,
                )
                sink(t, nn, orw, rec, hh, dd, wp)

        # ---------- context 1: P0 projection + AllGather0 + E0 + fused P1 ----
        with tile.TileContext(nc) as tc:
            with (
                tc.tile_pool(name="work", bufs=WBUFS) as wp,
                tc.tile_pool(name="psum", bufs=2, space="PSUM") as psp,
                tc.tile_pool(name="wconst", bufs=1) as wc,
                tc.tile_pool(name="gbuf", bufs=GBUFS) as gp,
                tc.tile_pool(name="psum1", bufs=2, space="PSUM") as psp1,
            ):
                w0_sb = wc.tile([P, KCH, f0 + 2 * h0], dt.bfloat16)
                nc.sync.dma_start(out=idx_sb[:], in_=eidx[:])
                nc.sync.dma_start(
                    out=w0_sb[:], in_=w0cat[:].rearrange("(c p) w -> p c w", p=P)
                )
                nc.sync.dma_start(out=w1_sb[:], in_=w1cat[:])
                nc.gpsimd.memset(er0_sb[:], 0.0)
                nc.gpsimd.memset(er1_sb[:], 0.0)
                make_identity(nc, ident[:])

                drow = wp.tile([1, row0], dt.bfloat16, tag="drow")
                nc.gpsimd.memset(drow[:], 0.0)
                nc.gpsimd.memset(drow[:, :2 * h0], -1e9)
                nc.sync.dma_start(out=tab0_sh[nsh:nsh + 1, :], in_=drow[:])
                drow1 = wp.tile([1, row1], dt.bfloat16, tag="drow1")
                nc.gpsimd.memset(drow1[:], 0.0)
                nc.gpsimd.memset(drow1[:, :2 * h1], -1e9)
                nc.sync.dma_start(out=tab1_sh[nsh:nsh + 1, :], in_=drow1[:])

                for t in range(ntt):
                    nn = min(P, nsh - t * P)
                    ps = psp.tile([P, f0 + 2 * h0], dt.float32, tag="proj")
                    xk = wp.tile([P, KCH, P], dt.bfloat16, tag="xk")
                    nc.sync.dma_start(
                        out=xk[:, :, :nn],
                        in_=xT[:, t * P:t * P + nn].rearrange("(c p) n -> p c n", p=P),
                    )
                    for kc in range(KCH):
                        nc.tensor.matmul(
                            ps[:nn, :], lhsT=xk[:, kc, :nn], rhs=w0_sb[:, kc, :],
                            start=(kc == 0), stop=(kc == KCH - 1),
                        )
                    row = wp.tile([P, row0], dt.bfloat16, tag="row")
                    nc.vector.tensor_copy(row[:nn, :h0], ps[:nn, f0:f0 + h0])
                    hi32 = wp.tile([P, h0], dt.float32, tag="hi32")
                    nc.vector.tensor_copy(hi32[:nn, :], row[:nn, :h0])
                    lo32 = wp.tile([P, h0], dt.float32, tag="lo32")
                    nc.vector.tensor_tensor(
                        out=lo32[:nn, :], in0=ps[:nn, f0:f0 + h0], in1=hi32[:nn, :],
                        op=OP.subtract,
                    )
                    nc.vector.tensor_copy(row[:nn, h0:2 * h0], lo32[:nn, :])
                    nc.scalar.activation(row[:nn, 2 * h0:2 * h0 + f0], ps[:nn, :f0], AF.Copy)
                    nc.vector.tensor_copy(er0_sb[:nn, t, :], ps[:nn, f0 + h0:])
                    nc.sync.dma_start(out=tab0_sh[t * P:t * P + nn, :], in_=row[:nn, :])

                if not SKIP_AG0:
                    nc.gpsimd.collective_compute(
                        "AllGather", OP.bypass, ins=[tab0_sh[:]], outs=[tab0[:]],
                        replica_groups=[list(range(NC))],
                    )
                else:
                    st = wp.tile([P, row0], dt.bfloat16, tag="stg")
                    for tt2 in range(ntt):
                        nnn = min(P, nsh - tt2 * P)
                        nc.sync.dma_start(out=st[:nnn, :], in_=tab0_sh[tt2*P:tt2*P+nnn, :])
                        nc.sync.dma_start(out=tab0[tt2*P:tt2*P+nnn, :], in_=st[:nnn, :])

                def sink0(t, nn, orw, rec, hh, dd, wp):
                    # h = elu(alpha-normalized aggregate), then project to the
                    # layer-1 table row immediately (P1 fused into E0).
                    x0 = wp.tile([P, f0], dt.float32, tag="x0")
                    nc.vector.tensor_tensor(
                        out=x0[:].rearrange("p (h d) -> p h d", h=hh),
                        in0=orw[:].rearrange("p (h d) -> p h d", h=hh),
                        in1=rec[:].to_broadcast([P, hh, dd]),
                        op=OP.mult,
                    )
                    relu = wp.tile([P, f0], dt.float32, tag="relu")
                    nc.vector.tensor_scalar_max(relu[:], x0[:], 0.0)
                    mneg = wp.tile([P, f0], dt.float32, tag="mneg")
                    nc.vector.tensor_scalar_min(mneg[:], x0[:], 0.0)
                    nc.scalar.activation(mneg[:], mneg[:], AF.Exp)
                    nc.vector.tensor_scalar(
                        out=mneg[:], in0=mneg[:], scalar1=-1.0, scalar2=0.0,
                        op0=OP.add, op1=OP.min,
                    )
                    hsb = wp.tile([P, f0], dt.float32, tag="hsb")
                    nc.vector.tensor_tensor(out=hsb[:], in0=relu[:], in1=mneg[:], op=OP.add)
                    hT_ps = psp1.tile([P, P], dt.float32, tag="hT")
                    nc.tensor.transpose(out=hT_ps[:], in_=hsb[:], identity=ident[:])
                    hT = wp.tile([P, P], dt.float32, tag="hTsb")
                    nc.vector.tensor_copy(hT[:], hT_ps[:])
                    ps1 = psp1.tile([P, f1 + 2 * h1], dt.float32, tag="proj1")
                    nc.tensor.matmul(
                        ps1[:nn, :], lhsT=hT[:, :nn], rhs=w1_sb[:], start=True, stop=True
                    )
                    row = wp.tile([P, row1], dt.bfloat16, tag="row1")
                    nc.vector.tensor_copy(row[:nn, :h1], ps1[:nn, f1:f1 + h1])
                    hi1 = wp.tile([P, h1], dt.float32, tag="hi1")
                    nc.vector.tensor_copy(hi1[:nn, :], row[:nn, :h1])
                    lo1 = wp.tile([P, h1], dt.float32, tag="lo1")
                    nc.vector.tensor_tensor(
                        out=lo1[:nn, :], in0=ps1[:nn, f1:f1 + h1], in1=hi1[:nn, :],
                        op=OP.subtract,
                    )
                    nc.vector.tensor_copy(row[:nn, h1:2 * h1], lo1[:nn, :])
                    nc.scalar.activation(row[:nn, 2 * h1:2 * h1 + f1], ps1[:nn, :f1], AF.Copy)
                    nc.vector.tensor_copy(er1_sb[:nn, t, :], ps1[:nn, f1 + h1:])
                    nc.sync.dma_start(out=tab1_sh[t * P:t * P + nn, :], in_=row[:nn, :])

                edge_phase(tc, wp, gp, tab0, h0, d0, row0, er0_sb, sink0, "0")

        # ---------- context 4: AllGather1 + E1 ----------
        with tile.TileContext(nc) as tc:
            with (
                tc.tile_pool(name="work", bufs=WBUFS) as wp,
                tc.tile_pool(name="gbuf", bufs=GBUFS) as gp,
            ):
                if not SKIP_AG1:
                    nc.gpsimd.collective_compute(
                        "AllGather", OP.bypass, ins=[tab1_sh[:]], outs=[tab1[:]],
                        replica_groups=[list(range(NC))],
                    )
                else:
                    st = wp.tile([P, row1], dt.bfloat16, tag="stg1")
                    for tt2 in range(ntt):
                        nnn = min(P, nsh - tt2 * P)
                        nc.sync.dma_start(out=st[:nnn, :row1], in_=tab1_sh[tt2*P:tt2*P+nnn, :])
                        nc.sync.dma_start(out=tab1[tt2*P:tt2*P+nnn, :], in_=st[:nnn, :row1])

                def sink1(t, nn, orw, rec, hh, dd, wp):
                    ov = wp.tile([P, hh * dd], dt.float32, tag="ov")
                    nc.vector.tensor_tensor(
                        out=ov[:].rearrange("p (h d) -> p h d", h=hh),
                        in0=orw[:].rearrange("p (h d) -> p h d", h=hh),
                        in1=rec[:].to_broadcast([P, hh, dd]),
                        op=OP.mult,
                    )
                    nc.sync.dma_start(out=out_d[t * P:t * P + nn, :], in_=ov[:nn, :])

                edge_phase(tc, wp, gp, tab1, h1, c1, row1, er1_sb, sink1, "1")

    nc.compile()
    return nc


_CACHE = {}


def build_cached(n_in, h0, d0, h1, c1, Kt, nsh, vsh, ntt):
    key = (n_in, h0, d0, h1, c1, nsh, vsh, ntt, tuple(Kt.tolist()))
    if key not in _CACHE:
        _CACHE[key] = _build_program(n_in, h0, d0, h1, c1, Kt, nsh, vsh, ntt)
    return _CACHE[key]


def make_in_maps(x, W0, al0, ar0, W1, al1, ar1, perm_c, idx2):
    n_in = x.shape[1]
    h0, d0 = al0.shape
    h1, c1 = al1.shape
    wl0 = np.einsum("ihd,hd->ih", W0.reshape(n_in, h0, d0), al0).astype(np.float32)
    wr0 = np.einsum("ihd,hd->ih", W0.reshape(n_in, h0, d0), ar0).astype(np.float32)
    wl1 = np.einsum("ihd,hd->ih", W1.reshape(h0 * d0, h1, c1), al1).astype(np.float32)
    wr1 = np.einsum("ihd,hd->ih", W1.reshape(h0 * d0, h1, c1), ar1).astype(np.float32)
    import ml_dtypes

    bf16 = ml_dtypes.bfloat16
    w0cat = np.ascontiguousarray(
        np.concatenate([W0, wl0, wr0], axis=1)
    ).astype(bf16)
    w1cat = np.ascontiguousarray(np.concatenate([W1, wl1, wr1], axis=1))
    return [
        {
            "xT": np.ascontiguousarray(x[perm_c[c]].T).astype(bf16),
            "eidx": np.ascontiguousarray(idx2[c]),
            "w0cat": w0cat,
            "w1cat": w1cat,
        }
        for c in range(NC)
    ]


LAST_EXEC_NS = None
LAST_MEAN_EXEC_NS = None


def kernel(x, src, dst, W0, al0, ar0, W1, al1, ar1):
    x = np.asarray(x, np.float32)
    src = np.asarray(src, np.int32)
    dst = np.asarray(dst, np.int32)
    W0 = np.asarray(W0, np.float32)
    al0 = np.asarray(al0, np.float32)
    ar0 = np.asarray(ar0, np.float32)
    W1 = np.asarray(W1, np.float32)
    al1 = np.asarray(al1, np.float32)
    ar1 = np.asarray(ar1, np.float32)

    n_nodes, n_in = x.shape
    h0, d0 = al0.shape
    h1, c1 = al1.shape

    perm_c, Kt, idx2, nsh, vsh, ntt = _host_shard(src, dst, n_nodes)
    nc = build_cached(n_in, h0, d0, h1, c1, Kt, nsh, vsh, ntt)
    in_maps = make_in_maps(x, W0, al0, ar0, W1, al1, ar1, perm_c, idx2)

    from concourse.bass_utils import run_bass_kernel_spmd

    trace = bool(int(os.environ.get("KERNEL_TRACE", "0")))
    res = run_bass_kernel_spmd(nc, in_maps, list(range(NC)), trace=trace)
    global LAST_EXEC_NS, LAST_MEAN_EXEC_NS
    LAST_EXEC_NS = res.exec_time_ns
    LAST_MEAN_EXEC_NS = res.mean_exec_time_ns
    out = np.empty((n_nodes, c1), np.float32)
    for c in range(NC):
        out[perm_c[c]] = res.results[c]["out"]
    return out

